# revision 7
# baseline (speedup 1.0000x reference)
"""Multi-head attention Trainium2 kernel (tensor-parallel over heads).

Per-core (head h): q/k/v projections, scores = q @ k^T / 8, softmax,
r3 = attn @ v, partial Z = r3 @ W0[64h:64h+64].  Host sums the 8 partial Z.

Precision strategy (validated on hardware: scaled absmax ~8e-5, rel l2
~2e-5 vs the fp32 ref):
  - q^T/k^T projections: native fp32 matmul (4 cyc/row, exact-ish);
    1/sqrt(D) folded into Wq on the host.  The stationary weights are
    duplicated along the output dim ([Wq|Wq], 128 cols) so the PSUM
    result lands on both partition halves directly - no SBUF partition-
    shift DMA is needed to build the hi/lo score operands.
  - scores: split q,k into fp16 hi + lo parts; one K=128 fp16 matmul
    computes q_hi*k_hi + q_lo*k_hi, two concurrent K=64 fp16 matmuls
    (opposite PE row-halves) add q_hi*k_lo.  22 mantissa bits combined,
    at fp16's 1 cyc/row with double-buffered weight loads.
  - softmax: per-chunk exp with chunk-local max (frees PSUM banks
    early), then a per-partition rescale by exp(cmax - rmax); division
    by the sum is deferred to the output projection.  attn is fp16.
    The rescale runs on the (otherwise idle) GpSimd engine so the
    Vector engine only carries the PSUM max-reductions.
  - v, AV, W0: fp32 / fp16 / fp32r (linear paths, below fp16 noise).
"""

import sys

import numpy as np

for _p in ("/opt/trn_rl_repo",):
    if _p not in sys.path:
        sys.path.insert(0, _p)

import concourse.bacc as bacc
import concourse.tile as tile
from concourse import mybir
from concourse.masks import make_identity
from contextlib import ExitStack

S, E, D, H = 4096, 512, 64, 8
P = 128
ST = S // P          # 32 query-row tiles
ET = E // P          # 4 embedding tiles
TT = S // P          # 32 key-row tiles
CH = 512             # free-dim chunk
NC_CH = S // CH      # 8 chunks
F32 = mybir.dt.float32
F32R = mybir.dt.float32r
F16 = mybir.dt.float16

_NC_CACHE = {}


def build_nc(repeat=1):
    nc = bacc.Bacc(None, target_bir_lowering=False)
    X = nc.declare_dram_parameter("X", [S, E], F32, isOutput=False)
    # q/k weights: fp16 hi+lo split AND duplicated along the output dim
    # ([W|W], 128 cols) on the host, so the projection matmuls are plain
    # fp16 (streaming weight loads) and the PSUM result lands on both
    # partition halves directly.
    Wqh = nc.declare_dram_parameter("Wqh", [E, 2 * D], F16, isOutput=False)
    Wql = nc.declare_dram_parameter("Wql", [E, 2 * D], F16, isOutput=False)
    Wkh = nc.declare_dram_parameter("Wkh", [E, 2 * D], F16, isOutput=False)
    Wkl = nc.declare_dram_parameter("Wkl", [E, 2 * D], F16, isOutput=False)
    Wv16 = nc.declare_dram_parameter("Wv16", [E, D], F16, isOutput=False)
    W0 = nc.declare_dram_parameter("W0", [D, D], F32, isOutput=False)
    Z = nc.declare_dram_parameter("Z", [S, D], F32, isOutput=True)

    with tile.TileContext(nc) as tc:
        for _ in range(repeat):
            with ExitStack() as ctx:
                body(ctx, tc, X, Wqh, Wql, Wkh, Wkl, Wv16, W0, Z)
    nc.finalize()
    return nc


def body(ctx, tc, X, Wqh, Wql, Wkh, Wkl, Wv16, W0, Z):
    nc = tc.nc

    const = ctx.enter_context(tc.tile_pool(name="const", bufs=1))
    identity = const.tile([P, P], F32)
    make_identity(nc, identity)

    wqh_sb = const.tile([P, ET, 2 * D], F16)
    wql_sb = const.tile([P, ET, 2 * D], F16)
    wkh_sb = const.tile([P, ET, 2 * D], F16)
    wkl_sb = const.tile([P, ET, 2 * D], F16)
    wv16 = const.tile([P, ET, D], F16)
    w0_sb = const.tile([D, D], F32)
    for w_dram, w_sb in ((Wqh, wqh_sb), (Wql, wql_sb),
                         (Wkh, wkh_sb), (Wkl, wkl_sb)):
        nc.sync.dma_start(
            out=w_sb, in_=w_dram.ap().rearrange("(t p) d -> p t d", p=P)
        )
    nc.sync.dma_start(
        out=wv16, in_=Wv16.ap().rearrange("(t p) d -> p t d", p=P)
    )
    nc.sync.dma_start(out=w0_sb, in_=W0.ap())

    # Persistent SBUF intermediates.  The score-matmul operands are fp16
    # hi + lo parts (11+11 mantissa bits): fp16 weights double-buffer in
    # the PE (fp32/f32r weights occupy both 16-bit weight planes, so every
    # weight switch stalls behind the previous matmul's drain; fp16 LDWs
    # are pulled ahead and the score matmuls stream at the systolic rate).
    big = ctx.enter_context(tc.tile_pool(name="big", bufs=1))
    qsp = big.tile([P, S], F16)      # rows 0-63: q_hi^T, rows 64-127: q_lo^T
    qh2 = big.tile([P, S], F16)      # rows 64-127: q_hi^T copy (row-packed MM2)
    ksph = big.tile([P, S], F16)     # k_hi^T duplicated on both halves
    # k_lo^T chunk-pairs: even chunk at rows 0-63, odd chunk at rows 64-127
    kspl = big.tile([P, CH, NC_CH // 2], F16)
    v16 = const.tile([P, TT, D], F16)      # v rows, fp16, t-tile major
    r3t = big.tile([D, S], F32)      # unnormalized r3^T
    inv_all = const.tile([P, ST], F32)     # 1/sumexp per s-tile column
    zfull = big.tile([P, ST, D], F32)      # normalized Z rows, batched store

    # ---- Stage A: load X, build X^T in SBUF via PE transposes ----
    with ExitStack() as sctx:
        xt_pool = sctx.enter_context(tc.tile_pool(name="xt", bufs=1))
        xTh = xt_pool.tile([P, ET, S], F16)  # X^T fp16 hi, e-tile major
        xTl = xt_pool.tile([P, ET, S], F16)  # X^T fp16 lo
        xload = sctx.enter_context(tc.tile_pool(name="xload", bufs=4))
        tp_ps = sctx.enter_context(
            tc.tile_pool(name="tp_ps", bufs=4, space="PSUM")
        )
        for i in range(ST):
            xn = xload.tile([P, E], F32, tag="xn")
            nc.sync.dma_start(out=xn, in_=X[i * P : (i + 1) * P, :])
            for j in range(ET):
                pt = tp_ps.tile([P, P], F32, tag="pt")
                nc.tensor.transpose(pt, xn[:, j * P : (j + 1) * P], identity)
                dh = xTh[:, j, i * P : (i + 1) * P]
                dl = xTl[:, j, i * P : (i + 1) * P]
                nc.scalar.copy(dh, pt)
                nc.vector.tensor_sub(dl, pt, dh)

        # ---- Stage B: projections ----
        pj_ps = sctx.enter_context(
            tc.tile_pool(name="pj_ps", bufs=3, space="PSUM")
        )
        vt_pool = sctx.enter_context(tc.tile_pool(name="vt", bufs=1))
        vT16 = vt_pool.tile([D, S], F16)

        # k first (scores need all of k but only the matching q chunk)
        for c in range(NC_CH):
            sl = slice(c * CH, (c + 1) * CH)
            pk = pj_ps.tile([P, CH], F32, tag="pj")
            for j in range(ET):
                nc.tensor.matmul(pk, wkh_sb[:, j, :], xTh[:, j, sl],
                                 start=(j == 0), stop=False)
                nc.tensor.matmul(pk, wkh_sb[:, j, :], xTl[:, j, sl],
                                 start=False, stop=False)
                nc.tensor.matmul(pk, wkl_sb[:, j, :], xTh[:, j, sl],
                                 start=False, stop=(j == ET - 1))
            # k_hi on both halves in one f32r write (rounds to FP22)
            nc.vector.tensor_copy(ksph[:, sl], pk)
            # lo part straight from PSUM: even chunks lanes 0-63, odd 64-127
            if c % 2 == 0:
                nc.vector.tensor_sub(kspl[0:D, :, c // 2],
                                     pk[0:D, :], ksph[0:D, sl])
            else:
                nc.vector.tensor_sub(kspl[D:P, :, c // 2],
                                     pk[D:P, :], ksph[D:P, sl])

        # v projection in fp16 straight from the hi part of X^T (v only
        # needs ~10 bits; the fp16 output noise dominates)
        for c in range(NC_CH):
            sl = slice(c * CH, (c + 1) * CH)
            pv = pj_ps.tile([D, CH], F32, tag="pj")
            for j in range(ET):
                nc.tensor.matmul(
                    pv, wv16[:, j, :], xTh[:, j, sl],
                    start=(j == 0), stop=(j == ET - 1),
                )
            nc.scalar.copy(vT16[:, sl], pv)

        # v^T [64, S] fp16 -> v16 [P, TT, D] in one xbar transpose.  Same
        # call shape as the attn transposes below, so the (partition, mid)
        # enumeration of the t axis matches for the AV contraction.
        nc.sync.dma_start_transpose(out=v16, in_=vT16)

        for c in range(NC_CH):
            sl = slice(c * CH, (c + 1) * CH)
            pq = pj_ps.tile([P, CH], F32, tag="pj")
            for j in range(ET):
                nc.tensor.matmul(pq, wqh_sb[:, j, :], xTh[:, j, sl],
                                 start=(j == 0), stop=False)
                nc.tensor.matmul(pq, wqh_sb[:, j, :], xTl[:, j, sl],
                                 start=False, stop=False)
                nc.tensor.matmul(pq, wql_sb[:, j, :], xTh[:, j, sl],
                                 start=False, stop=(j == ET - 1))
            nc.vector.tensor_copy(qsp[0:D, sl], pq[0:D, :])
            nc.vector.tensor_copy(qh2[D:P, sl], pq[D:P, :])
            nc.vector.tensor_sub(qsp[D:P, sl], pq[D:P, :], qh2[D:P, sl])


    # ---- Stage C/D/E: scores -> softmax -> AV -> W0, per s-tile ----
    # PSUM: 3 rotating 2-bank score slots + 1 bank for AV + 1 bank for W0,
    # so the AV/W0 matmuls never steal slots from the next tile's scores.
    pbank = ctx.enter_context(tc.tile_pool(name="pbank", bufs=3, space="PSUM"))
    rp_ps = ctx.enter_context(tc.tile_pool(name="rp_ps", bufs=1, space="PSUM"))
    zp_ps = ctx.enter_context(tc.tile_pool(name="zp_ps", bufs=1, space="PSUM"))
    stats = ctx.enter_context(tc.tile_pool(name="stats", bufs=6))
    attn_pool = ctx.enter_context(tc.tile_pool(name="attn", bufs=4))
    attnT_pool = ctx.enter_context(tc.tile_pool(name="attnT", bufs=2))

    NP = NC_CH // 2              # 4 chunk-pairs (2 PSUM banks each)
    GR = 4                       # s-tiles per AV group (AV free dim = 512)

    # Software-pipelined tile loop: iteration i emits the HEAD of tile i
    # (score matmuls, chunk maxes, exps) and then the TAIL of tile i-1
    # (row max, rescales, transpose, sum stats).  With strict per-engine
    # FIFOs this keeps tile i's reduces/exps ahead of tile (i-1)'s tail in
    # the Vector/Scalar queues, so the cross-engine fvec/rescale ping-pong
    # no longer head-of-line-blocks the next tile.  AV for group g is
    # emitted one further iteration later (i == 4g+5) so its attn
    # transposes have a full tile of slack.
    attnT_tiles = {}
    prev = None
    for i in range(ST + 2):
        if i < ST:
            ssl = slice(i * P, (i + 1) * P)
            cmax = stats.tile([P, NP], F32, tag="cmax")   # holds -chunkmax
            acc = stats.tile([P, NP], F32, tag="acc")
            attn_i = attn_pool.tile([P, S], F16, tag="attn")
            for c in range(NP):
                pb = pbank.tile([P, 2 * CH], F32, tag="pb")
                e0 = slice(2 * c * CH, (2 * c + 1) * CH)
                e1 = slice((2 * c + 1) * CH, (2 * c + 2) * CH)
                nc.tensor.matmul(pb[:, 0:CH], qsp[:, ssl], ksph[:, e0],
                                 start=True, stop=False)
                nc.tensor.matmul(pb[:, CH : 2 * CH], qsp[:, ssl],
                                 ksph[:, e1], start=True, stop=False)
                nc.tensor.matmul(pb[:, 0:CH], qsp[0:D, ssl],
                                 kspl[0:D, :, c], start=False, stop=True)
                nc.tensor.matmul(pb[:, CH : 2 * CH], qh2[D:P, ssl],
                                 kspl[D:P, :, c], start=False, stop=True)
                # negated chunk max straight from the reduce: serves as the
                # exp bias with no intermediate negation op
                nc.vector.reduce_max(out=cmax[:, c : c + 1], in_=pb,
                                     axis=mybir.AxisListType.X, negate=True)
                # q was pre-scaled by 1/8, so psum is final scores
                nc.scalar.activation(
                    out=attn_i[:, 2 * c * CH : (2 * c + 2) * CH], in_=pb,
                    func=mybir.ActivationFunctionType.Exp,
                    bias=cmax[:, c : c + 1], scale=1.0,
                    accum_out=acc[:, c : c + 1],
                )
            cur = (cmax, acc, attn_i)
        else:
            cur = None

        if prev is not None:
            t = i - 1
            cmax, acc, attn_i = prev
            g = t // GR
            if t % GR == 0:
                attnT_g_new = attnT_pool.tile([P, TT, GR * P], F16,
                                              tag="attnT")
                attnT_tiles[g] = attnT_g_new
            # cmax holds -chunkmax, so -rowmax = min over it
            nbias = stats.tile([P, 1], F32, tag="nbias")
            nc.vector.tensor_reduce(out=nbias, in_=cmax,
                                    axis=mybir.AxisListType.X,
                                    op=mybir.AluOpType.min)
            fvec = stats.tile([P, NP], F32, tag="fvec")
            nc.scalar.activation(fvec, cmax,
                                 func=mybir.ActivationFunctionType.Exp,
                                 bias=nbias, scale=-1.0)
            # rescale each fp16 chunk-pair (3 on Vector, 1 on Scalar)
            for c in range(NP):
                sl2 = slice(2 * c * CH, (2 * c + 2) * CH)
                if c == NP - 1:
                    nc.scalar.activation(
                        attn_i[:, sl2], attn_i[:, sl2],
                        func=mybir.ActivationFunctionType.Copy,
                        scale=fvec[:, c : c + 1])
                else:
                    nc.vector.tensor_scalar_mul(attn_i[:, sl2],
                                                attn_i[:, sl2],
                                                fvec[:, c : c + 1])
            nc.sync.dma_start_transpose(
                out=attnT_tiles[g][:, :, (t % GR) * P : (t % GR + 1) * P],
                in_=attn_i,
            )
            accw = stats.tile([P, NP], F32, tag="accw")
            nc.vector.tensor_mul(accw, acc, fvec)
            sm = stats.tile([P, 1], F32, tag="sm")
            nc.vector.tensor_reduce(out=sm, in_=accw,
                                    axis=mybir.AxisListType.X,
                                    op=mybir.AluOpType.add)
            nc.vector.reciprocal(inv_all[:, t : t + 1], sm)
        prev = cur

        # AV + output projection for group g once its last tile's tail
        # (emitted at iteration 4g+4) is in the queues: i == 4g+5
        if i >= GR + 1 and (i - GR - 1) % GR == 0:
            g = (i - GR - 1) // GR
            gsl = slice(g * GR * P, (g + 1) * GR * P)
            attnT_g = attnT_tiles.pop(g)
            rp = rp_ps.tile([D, GR * P], F32, tag="rp")
            for j in range(TT):
                nc.tensor.matmul(rp, v16[:, j, :], attnT_g[:, j, :],
                                 start=(j == 0), stop=(j == TT - 1))
            nc.scalar.copy(r3t[:, gsl], rp)

            # output projection + normalization for the 4 finished s-tiles
            zp_full = zp_ps.tile([P, GR * D], F32, tag="zp")
            for gi in range(GR):
                si = g * GR + gi
                zp = zp_full[:, gi * D : (gi + 1) * D]
                nc.tensor.matmul(zp, r3t[:, si * P : (si + 1) * P],
                                 w0_sb, start=True, stop=True)
                nc.vector.tensor_scalar_mul(zfull[:, si, :], zp,
                                            inv_all[:, si : si + 1])

    # batched output store: one 1 MB DMA instead of 32 x 32 KB
    nc.sync.dma_start(out=Z.ap().rearrange("(t p) d -> p t d", p=P),
                      in_=zfull)


def _get_nc():
    if "nc" not in _NC_CACHE:
        _NC_CACHE["nc"] = build_nc()
    return _NC_CACHE["nc"]


def _hilo16(w):
    hi = w.astype(np.float16)
    lo = (w - hi.astype(np.float32)).astype(np.float16)
    return hi, lo


def make_in_maps(X, W_q, W_k, W_v, W_0):
    in_maps = []
    for h in range(H):
        # 1/sqrt(D) folded into Wq so scores land pre-scaled in PSUM
        qh, ql = _hilo16(np.asarray(W_q[h], dtype=np.float32) * np.float32(0.125))
        kh, kl = _hilo16(np.asarray(W_k[h], dtype=np.float32))
        dup = lambda w: np.ascontiguousarray(np.concatenate([w, w], axis=1))
        in_maps.append({
            "X": np.ascontiguousarray(X, dtype=np.float32),
            "Wqh": dup(qh), "Wql": dup(ql),
            "Wkh": dup(kh), "Wkl": dup(kl),
            "Wv16": np.ascontiguousarray(W_v[h], dtype=np.float16),
            "W0": np.ascontiguousarray(W_0[h * D : (h + 1) * D, :], dtype=np.float32),
        })
    return in_maps


def kernel(X, W_q, W_k, W_v, W_0):
    from concourse.bass_utils import run_bass_kernel_spmd

    nc = _get_nc()
    res = run_bass_kernel_spmd(nc, make_in_maps(X, W_q, W_k, W_v, W_0),
                               list(range(H)))
    Zp = [res.results[h]["Z"] for h in range(H)]
    return np.sum(np.stack(Zp, axis=0), axis=0, dtype=np.float32)
